# revision 62
# baseline (speedup 1.0000x reference)
"""GCN 3-layer kernel for Trainium2, 8 NeuronCores.

Full inputs in, full output out. Internally: dst-sharded SpMM via dma_gather
(bf16 tables, int16 window-relative indices) + one-hot matmul segment sum,
AllGather between layers, dense transforms per dst tile.
"""
import sys
sys.path.insert(0, "/opt/trn_rl_repo")
import os
import numpy as np
import ml_dtypes

import concourse.bass as bass
import concourse.bacc as bacc
import concourse.mybir as mybir
import concourse.tile as tile
from concourse.bass_utils import run_bass_kernel_spmd

P = 128
NCORES = 8
N_NODES = 100000
SHARD = N_NODES // NCORES           # 12500
TILES = (SHARD + P - 1) // P        # 98
SHARD_PAD = TILES * P               # 12544
PAD = SHARD_PAD - SHARD             # 44
NSTAGED = NCORES * SHARD_PAD        # 100352
NSLICE = 7                          # AllGather slices (1 batch each)
SL = SHARD_PAD // NSLICE            # 1792 rows per slice per core
IN_F, HID, OUT_F = 50, 256, 121
F1 = 64                             # x' padded width (fp8 msg, 64B rows)
F2 = 256                            # h1' width (fp8, 256B rows)
F3 = 256                            # t3' padded table width (fp8, 256B rows)
F3O = 128                           # L3 accumulator / output staging width
PIECES = 4
TB = 14                             # tiles per gather batch
BATCHES = TILES // TB               # 7
# source windows over staged-id domain [0, 100352): width 32768 each
WBASE = [0, 22528, 45056, 67584]

bf16 = mybir.dt.bfloat16
f32 = mybir.dt.float32
f8 = mybir.dt.float8e4
i16 = mybir.dt.int16

_CACHE = {}


def _ensure_ntff_hook():
    """Provide antenv.axon_hooks if the image lacks it, so trace=True works."""
    try:
        from antenv.axon_hooks import get_axon_ntff_profile_hook  # noqa: F401
        return
    except ImportError:
        pass
    import types
    mod = types.ModuleType("antenv.axon_hooks")
    mod._hook = None

    def set_axon_ntff_profile_hook(h):
        mod._hook = h

    def get_axon_ntff_profile_hook():
        return mod._hook

    mod.set_axon_ntff_profile_hook = set_axon_ntff_profile_hook
    mod.get_axon_ntff_profile_hook = get_axon_ntff_profile_hook
    sys.modules["antenv.axon_hooks"] = mod
    try:
        import antenv
        antenv.axon_hooks = mod
    except ImportError:
        pass
    try:
        from trn_agent_boot.trn_boot import _ntff_profile_via_ctypes
        h = _ntff_profile_via_ctypes("/opt/axon/libaxon_pjrt.so")
        if h is not None:
            mod._hook = h
    except Exception:
        pass


def _staged(v):
    """node id -> staged table id, slice-major so AllGather slices land
    contiguously: [slice][core][row-within-slice]."""
    c = v // SHARD
    r = v % SHARD
    s = r // SL
    return s * (NCORES * SL) + c * SL + (r % SL)


def _preprocess(edge_index):
    """Build per-core gather/segment streams with variable per-(tile,piece)
    chunk capacities (core-uniform). Returns dict of host arrays + plan."""
    src = np.asarray(edge_index[0], dtype=np.int64)
    dst = np.asarray(edge_index[1], dtype=np.int64)
    deg = (np.bincount(dst, minlength=N_NODES) + 1).astype(np.float64)
    dinv = (1.0 / np.sqrt(deg)).astype(np.float32)

    # per (core, tile): sorted source list + seg values
    ss_all = {}
    sg_all = {}
    e_ct = np.zeros((NCORES, TILES), dtype=np.int64)
    for c in range(NCORES):
        base = c * SHARD
        m = (dst >= base) & (dst < base + SHARD)
        sp = _staged(src[m])
        dl = dst[m] - base
        tl = dl >> 7
        seg = (dl & 127).astype(np.float32)
        key = tl * (1 << 17) + sp
        o = np.argsort(key, kind="stable")
        sp, seg, tl = sp[o], seg[o], tl[o]
        tcnt = np.bincount(tl, minlength=TILES)
        toff = np.concatenate([[0], np.cumsum(tcnt)])
        for t in range(TILES):
            ss_all[c, t] = sp[toff[t]:toff[t + 1]]
            sg_all[c, t] = seg[toff[t]:toff[t + 1]]
            e_ct[c, t] = toff[t + 1] - toff[t]

    # core-uniform per-(tile,piece) capacities n_tk (chunks of 128 slots)
    def cuts_for(ss, n_tk):
        cap = [P * n for n in n_tk]
        e = len(ss)
        cuts = [0]
        for k in range(PIECES):
            if k == PIECES - 1:
                nxt = e
            else:
                lo = int(np.searchsorted(ss, WBASE[k + 1]))
                hi = int(np.searchsorted(ss, WBASE[k] + 32768))
                need = e - sum(cap[k + 1:])
                nxt = min(hi, max(lo, need, cuts[-1]), cuts[-1] + cap[k])
                nxt = max(nxt, need)
                if nxt > hi or nxt < lo or nxt < cuts[-1]:
                    return None
            if nxt - cuts[-1] > cap[k]:
                return None
            cuts.append(nxt)
        return cuts

    n_tk_all = []
    for t in range(TILES):
        ct = int(-(-e_ct[:, t].max() // P))
        while True:
            bn, rem = ct // PIECES, ct % PIECES
            n_tk = [bn + (k < rem) for k in range(PIECES)]
            if all(cuts_for(ss_all[c, t], n_tk) is not None
                   for c in range(NCORES)):
                break
            ct += 1
        n_tk_all.append(n_tk)

    # per (batch, piece): stream length; per (batch): chunk column layout
    nk_bk = [[sum(n_tk_all[b * TB + ti][k] for ti in range(TB))
              for k in range(PIECES)] for b in range(BATCHES)]
    chb_b = [sum(nk) for nk in nk_bk]
    CHB = max(chb_b)
    WCOL = max(max(nk) for nk in nk_bk) * P // 16
    # column index of chunk (b, k, ti, cc) in the per-batch stream
    cols = []  # cols[b][ti] = list of column indices (piece-major)
    for b in range(BATCHES):
        offk = np.concatenate([[0], np.cumsum(nk_bk[b])])
        bt = []
        for ti in range(TB):
            t = b * TB + ti
            cl = []
            for k in range(PIECES):
                pre = sum(n_tk_all[b * TB + tj][k] for tj in range(ti))
                for cc in range(n_tk_all[t][k]):
                    cl.append(int(offk[k]) + pre + cc)
            bt.append(cl)
        cols.append(bt)

    widx = np.zeros((NCORES, BATCHES, PIECES, P, WCOL), dtype=np.int16)
    segar = np.full((NCORES, BATCHES, P, CHB), -1.0, dtype=np.float32)
    sp_slots = np.zeros((NCORES, BATCHES, CHB * P), dtype=np.int64)
    dinv_t = np.ones((NCORES, P, TILES), dtype=np.float32)
    for c in range(NCORES):
        base = c * SHARD
        for b in range(BATCHES):
            offk = np.concatenate([[0], np.cumsum(nk_bk[b])])
            for k in range(PIECES):
                nk = nk_bk[b][k]
                stream = np.zeros(nk * P, dtype=np.int16)
                segstr = np.full((nk, P), -1.0, dtype=np.float32)
                spstr = np.full(nk * P, WBASE[k], dtype=np.int64)
                pos = 0
                for ti in range(TB):
                    t = b * TB + ti
                    ss = ss_all[c, t]
                    sg = sg_all[c, t]
                    cuts = cuts_for(ss, n_tk_all[t])
                    a, bb = cuts[k], cuts[k + 1]
                    n = bb - a
                    rel = ss[a:bb] - WBASE[k]
                    assert (rel >= 0).all() and (rel < 32768).all()
                    cap = n_tk_all[t][k] * P
                    stream[pos:pos + n] = rel.astype(np.int16)
                    spstr[pos:pos + n] = ss[a:bb]
                    fl = segstr.reshape(-1)
                    fl[pos:pos + n] = sg[a:bb]
                    pos += cap
                w = stream.reshape(-1, 16).T
                widx[c, b, k, :, :nk * P // 16] = np.tile(w, (8, 1))
                segar[c, b, :, offk[k]:offk[k + 1]] = segstr.T
                sp_slots[c, b, offk[k] * P:offk[k + 1] * P] = spstr
        for t in range(TILES):
            lo = t * P
            n = max(0, min(P, SHARD - lo))
            if n > 0:
                dinv_t[c, :n, t] = dinv[base + lo:base + lo + n]
    plan = dict(nk_bk=nk_bk, chb_b=chb_b, CHB=CHB, WCOL=WCOL, cols=cols)
    return dict(widx=widx, segar=segar, dinv_t=dinv_t, dinv=dinv,
                sp_slots=sp_slots, plan=plan)


def _build_msg1(pre, xs):
    """Host pre-gather of the layer-1 message stream: slot i of batch b lands
    at msg1[b][i%128, i//128] (dma_gather output layout)."""
    CHB = pre["plan"]["CHB"]
    sp = pre["sp_slots"]  # [NCORES, BATCHES, CHB*P]
    msg1 = np.zeros((NCORES, BATCHES, P, CHB, F1), dtype=xs.dtype)
    for c in range(NCORES):
        for b in range(BATCHES):
            rows = xs[sp[c, b]]  # [CHB*P, F1]
            msg1[c, b] = rows.reshape(CHB, P, F1).transpose(1, 0, 2)
    return msg1


def _build_program(plan):
    """Build the (core-uniform) Bass program from the chunk plan."""
    nbatch = int(os.environ.get("KERNEL_NBATCH", str(BATCHES)))
    CH_B = plan["CHB"]
    WCOL = plan["WCOL"]
    nk_bk = plan["nk_bk"]
    cols = plan["cols"]

    nq = int(os.environ.get("KERNEL_NQ", "1"))
    nc = bacc.Bacc("TRN2", target_bir_lowering=False, debug=False,
                   enable_asserts=False, num_devices=NCORES,
                   num_swdge_queues=nq)

    t_msg1 = nc.dram_tensor("msg1", [BATCHES, P, CH_B, F1], f8,
                            kind="ExternalInput")
    t_widx = nc.dram_tensor("widx", [BATCHES, P, PIECES * WCOL], i16,
                            kind="ExternalInput")
    t_bmat = nc.dram_tensor("bmat", [BATCHES, P, CH_B * P], f8,
                            kind="ExternalInput")
    t_dinv = nc.dram_tensor("dinv_t", [P, TILES], f32, kind="ExternalInput")
    t_w1 = nc.dram_tensor("w1", [F1, HID], bf16, kind="ExternalInput")
    t_w2 = nc.dram_tensor("w2", [HID, HID], bf16, kind="ExternalInput")
    t_w3 = nc.dram_tensor("w3", [HID, F3], bf16, kind="ExternalInput")
    t_b1 = nc.dram_tensor("b1b", [P, HID], f32, kind="ExternalInput")
    t_b2 = nc.dram_tensor("b2b", [P, HID], f32, kind="ExternalInput")
    t_b3 = nc.dram_tensor("b3b", [P, F3O], f32, kind="ExternalInput")
    t_ident = nc.dram_tensor("ident", [P, P], f32, kind="ExternalInput")
    t_xself = nc.dram_tensor("xself", [SHARD_PAD, F1], bf16,
                             kind="ExternalInput")
    t_out = nc.dram_tensor("out_shard", [SHARD_PAD, F3O], f32,
                           kind="ExternalOutput")
    dbg = os.environ.get("KERNEL_DEBUG", "0") == "1"
    dbg2 = os.environ.get("KERNEL_DEBUG2", "0") == "1"
    if dbg2:
        t_dbga = nc.dram_tensor("dbg_acc", [SHARD_PAD, F1], f32,
                                kind="ExternalOutput")
    if dbg:
        t_dbg1 = nc.dram_tensor("dbg_h1", [SHARD_PAD, F2], f32,
                                kind="ExternalOutput")
        t_dbg3 = nc.dram_tensor("dbg_t3", [SHARD_PAD, F3O], f32,
                                kind="ExternalOutput")

    with tile.TileContext(nc) as tc:
        with (
            tc.tile_pool(name="consts", bufs=1) as consts,
            tc.tile_pool(name="stream", bufs=3) as stream,
            tc.tile_pool(name="msgp", bufs=2) as msgp,
            tc.tile_pool(name="work", bufs=3) as work,
            tc.tile_pool(name="bpool", bufs=1) as bpool,
            tc.tile_pool(name="psum", bufs=3, space="PSUM") as psum,
            tc.tile_pool(name="psumd", bufs=2, space="PSUM") as psumd,
            tc.tile_pool(name="dram", bufs=1, space="DRAM") as dram,
        ):
            ident_t = consts.tile([P, P], f32)
            nc.sync.dma_start(out=ident_t[:], in_=t_ident[:])
            dinv_c = consts.tile([P, TILES], f32)
            nc.sync.dma_start(out=dinv_c[:], in_=t_dinv[:])
            w1_t = consts.tile([F1, HID], bf16)
            nc.sync.dma_start(out=w1_t[:], in_=t_w1[:])
            w2_ts = []
            for kk in range(2):
                wt = consts.tile([P, HID], bf16, name=f"w2t{kk}")
                nc.sync.dma_start(out=wt[:], in_=t_w2[kk * P:(kk + 1) * P, :])
                w2_ts.append(wt)
            w3_ts = []
            for kk in range(2):
                wt = consts.tile([P, F3], bf16, name=f"w3t{kk}")
                nc.sync.dma_start(out=wt[:], in_=t_w3[kk * P:(kk + 1) * P, :])
                w3_ts.append(wt)
            b1_t = consts.tile([P, HID], f32)
            nc.sync.dma_start(out=b1_t[:], in_=t_b1[:])
            b2_t = consts.tile([P, HID], f32)
            nc.sync.dma_start(out=b2_t[:], in_=t_b2[:])
            b3_t = consts.tile([P, F3O], f32)
            nc.sync.dma_start(out=b3_t[:], in_=t_b3[:])

            h1_stage = dram.tile([SHARD_PAD, F2], f8)
            h1_full = dram.tile([NSTAGED, F2], f8)
            t3_stage = dram.tile([SHARD_PAD, F3], f8)
            t3_full = dram.tile([NSTAGED, F3], f8)
            # per-slice AllGather landing buffers (Shared = peer-writable;
            # each is written by exactly one collective)
            ag_h1 = [dram.tile([SL * NCORES, F2], f8, addr_space="Shared",
                               name=f"agh1_{s}") for s in range(NSLICE)]
            ag_t3 = [dram.tile([SL * NCORES, F3], f8, addr_space="Shared",
                               name=f"agt3_{s}") for s in range(NSLICE)]

            def spmm_layer(layer, table_ap, elem, accw, tail_fn,
                           after_batch=None):
                use_dr = layer in (2, 3)
                dr = mybir.MatmulPerfMode.DoubleRow
                for b in range(nbatch):
                    offk = [0]
                    for k in range(PIECES):
                        offk.append(offk[-1] + nk_bk[b][k])

                    def piece_of(g):
                        kg = 0
                        while kg < PIECES - 1 and g >= offk[kg + 1]:
                            kg += 1
                        return kg

                    bm = bpool.tile([P, CH_B * P], f8, tag="bm")
                    nc.sync.dma_start(out=bm[:], in_=t_bmat[b])
                    msg = msgp.tile([P, CH_B, elem], f8, tag="msg")
                    if layer == 1:
                        nc.sync.dma_start(out=msg[:], in_=t_msg1[b])
                    else:
                        it = stream.tile([P, PIECES * WCOL], i16, tag="idx")
                        nc.sync.dma_start(out=it[:], in_=t_widx[b])
                        sp_mode = os.environ.get("KERNEL_SP", "0") == "1"
                        for k in range(PIECES):
                            nk = nk_bk[b][k]
                            nc.gpsimd.dma_gather(
                                msg[:, offk[k]:offk[k + 1], :],
                                table_ap[WBASE[k]:WBASE[k] + 32768, :],
                                it[:, k * WCOL:k * WCOL + nk * P // 16],
                                nk * P, nk * P, elem,
                                single_packet=sp_mode,
                                queue_num=k % nq,
                            )
                    for ti in range(TB):
                        t = b * TB + ti
                        cl = cols[b][ti]
                        acc = psum.tile([P, accw], f32, tag="acc")
                        items = []      # (kg, gl, g, pair)
                        q = 0
                        while q < len(cl):
                            g = cl[q]
                            if (use_dr and q + 1 < len(cl)
                                    and cl[q + 1] == g + 1):
                                items.append((0, 0, g, True))
                                q += 2
                            else:
                                items.append((0, 0, g, False))
                                q += 1
                        for j, (kg, gl, g, pair) in enumerate(items):
                            if pair:
                                nc.tensor.matmul(
                                    out=acc[:],
                                    lhsT=bm[:, g * P:(g + 2) * P]
                                    .rearrange("p (k m) -> p k m", k=2),
                                    rhs=msg[:, g:g + 2, :accw],
                                    start=(j == 0),
                                    stop=(j == len(items) - 1),
                                    perf_mode=dr,
                                )
                            else:
                                nc.tensor.matmul(
                                    out=acc[:],
                                    lhsT=bm[:, g * P:(g + 1) * P],
                                    rhs=msg[:, g:g + 1, :accw],
                                    start=(j == 0),
                                    stop=(j == len(items) - 1),
                                )
                        tail_fn(t, acc)
                    if after_batch is not None:
                        after_batch(b)

            def dense(lhs_sbuf_f32, wts, fout, kw=P):
                """lhs [P, nk*kw] f32 sbuf (node rows) -> psum [P, fout]"""
                nk = len(wts)
                o2 = psumd.tile([P, fout], f32, tag="dense")
                for kk in range(nk):
                    tp = psum.tile([P, P], f32, tag="tp")
                    nc.tensor.transpose(
                        out=tp[:kw, :],
                        in_=lhs_sbuf_f32[:, kk * kw:(kk + 1) * kw],
                        identity=ident_t[:])
                    lt = work.tile([P, P], bf16, tag="lt")
                    nc.scalar.activation(
                        out=lt[:kw, :], in_=tp[:kw, :],
                        func=mybir.ActivationFunctionType.Copy)
                    nc.tensor.matmul(
                        out=o2[:], lhsT=lt[:kw, :], rhs=wts[kk][:, :fout],
                        start=(kk == 0), stop=(kk == nk - 1))
                return o2

            def tail1(t, acc):
                if dbg2:
                    af = work.tile([P, F1], f32, tag="af")
                    nc.vector.tensor_copy(out=af[:], in_=acc[:])
                    nc.sync.dma_start(out=t_dbga[t * P:(t + 1) * P, :],
                                      in_=af[:])
                st = work.tile([P, F1], bf16, tag="selft")
                nc.sync.dma_start(out=st[:], in_=t_xself[t * P:(t + 1) * P, :])
                agg0 = work.tile([P, F1], f32, tag="agg0")
                nc.vector.scalar_tensor_tensor(
                    out=agg0[:], in0=acc[:], scalar=dinv_c[:, t:t + 1],
                    in1=st[:], op0=mybir.AluOpType.mult,
                    op1=mybir.AluOpType.add)
                o2 = dense(agg0, [w1_t], HID, kw=F1)
                s1 = work.tile([P, HID], f32, tag="s1")
                nc.vector.tensor_tensor(out=s1[:], in0=o2[:], in1=b1_t[:],
                                        op=mybir.AluOpType.add)
                h1t = work.tile([P, HID], f8, tag="h1t")
                nc.scalar.activation(
                    out=h1t[:], in_=s1[:],
                    func=mybir.ActivationFunctionType.Relu,
                    scale=dinv_c[:, t:t + 1])
                nc.sync.dma_start(out=h1_stage[t * P:(t + 1) * P, :], in_=h1t[:])
                if dbg:
                    h1f = work.tile([P, HID], f32, tag="h1f")
                    nc.vector.tensor_copy(out=h1f[:], in_=h1t[:])
                    nc.sync.dma_start(out=t_dbg1[t * P:(t + 1) * P, :],
                                      in_=h1f[:])

            def tail2(t, acc):
                st8 = work.tile([P, HID], f8, tag="selft2")
                nc.sync.dma_start(out=st8[:],
                                  in_=h1_stage[t * P:(t + 1) * P, :])
                st = work.tile([P, HID], f32, tag="selfc2")
                nc.scalar.activation(
                    out=st[:], in_=st8[:],
                    func=mybir.ActivationFunctionType.Copy,
                    scale=dinv_c[:, t:t + 1])
                agg0 = work.tile([P, HID], f32, tag="agg02")
                nc.vector.scalar_tensor_tensor(
                    out=agg0[:], in0=acc[:], scalar=dinv_c[:, t:t + 1],
                    in1=st[:], op0=mybir.AluOpType.mult,
                    op1=mybir.AluOpType.add)
                o2 = dense(agg0, w2_ts, HID)
                s2 = work.tile([P, HID], f32, tag="s1")
                nc.vector.tensor_tensor(out=s2[:], in0=o2[:], in1=b2_t[:],
                                        op=mybir.AluOpType.add)
                h2t = work.tile([P, HID], f32, tag="h2t")
                nc.scalar.activation(
                    out=h2t[:], in_=s2[:],
                    func=mybir.ActivationFunctionType.Relu,
                    scale=dinv_c[:, t:t + 1])
                o3 = dense(h2t, w3_ts, F3)
                t3t = work.tile([P, F3], f8, tag="t3t")
                nc.scalar.activation(
                    out=t3t[:], in_=o3[:],
                    func=mybir.ActivationFunctionType.Copy)
                nc.sync.dma_start(out=t3_stage[t * P:(t + 1) * P, :], in_=t3t[:])
                if dbg:
                    t3f = work.tile([P, F3O], f32, tag="t3f")
                    nc.vector.tensor_copy(out=t3f[:], in_=t3t[:, :F3O])
                    nc.sync.dma_start(out=t_dbg3[t * P:(t + 1) * P, :],
                                      in_=t3f[:])

            def tail3(t, acc):
                st8 = work.tile([P, F3O], f8, tag="selft3")
                nc.sync.dma_start(
                    out=st8[:], in_=t3_stage[t * P:(t + 1) * P, :F3O])
                st = work.tile([P, F3O], f32, tag="selfc3")
                nc.scalar.activation(
                    out=st[:], in_=st8[:],
                    func=mybir.ActivationFunctionType.Copy,
                    scale=dinv_c[:, t:t + 1])
                agg0 = work.tile([P, F3O], f32, tag="agg03")
                nc.vector.scalar_tensor_tensor(
                    out=agg0[:], in0=acc[:], scalar=dinv_c[:, t:t + 1],
                    in1=st[:], op0=mybir.AluOpType.mult,
                    op1=mybir.AluOpType.add)
                res = work.tile([P, F3O], f32, tag="res")
                nc.vector.tensor_tensor(
                    out=res[:], in0=agg0[:], in1=b3_t[:],
                    op=mybir.AluOpType.add)
                nc.sync.dma_start(out=t_out[t * P:(t + 1) * P, :], in_=res[:])

            nlayer = int(os.environ.get("KERNEL_NLAYER", "3"))

            def slice_ag(stage, full, bufs):
                bps = BATCHES // NSLICE
                def after_batch(b):
                    if (b + 1) % bps == 0:
                        s = (b + 1) // bps - 1
                        nc.gpsimd.collective_compute(
                            "AllGather", mybir.AluOpType.bypass,
                            replica_groups=[list(range(NCORES))],
                            ins=[stage[s * SL:(s + 1) * SL, :].opt()],
                            outs=[bufs[s][:].opt()])
                        nc.sync.dma_start(
                            out=full[s * SL * NCORES:(s + 1) * SL * NCORES,
                                     :],
                            in_=bufs[s][:])
                return after_batch

            spmm_layer(1, None, F1, F1, tail1,
                       after_batch=(slice_ag(h1_stage, h1_full, ag_h1)
                                    if nlayer >= 2 else None))
            if nlayer >= 2:
                spmm_layer(2, h1_full, F2, F2, tail2,
                           after_batch=(slice_ag(t3_stage, t3_full, ag_t3)
                                        if nlayer >= 3 else None))
            if nlayer >= 3:
                spmm_layer(3, t3_full, F3, F3O, tail3)

    nc.compile()
    return nc


def kernel(x, edge_index, W1, b1, W2, b2, W3, b3):
    x = np.asarray(x, dtype=np.float32)
    pre = _preprocess(np.asarray(edge_index))
    plan = pre["plan"]
    key = tuple(tuple(nk) for nk in plan["nk_bk"])

    if key not in _CACHE:
        _CACHE[key] = _build_program(plan)
    nc = _CACHE[key]

    dinv = pre["dinv"]
    xs = np.zeros((NSTAGED, F1), dtype=np.float32)
    xp = dinv[:, None] * x                      # [N, 50]
    xs[_staged(np.arange(N_NODES)), :IN_F] = xp
    xs = xs.astype(ml_dtypes.bfloat16)

    w1p = np.zeros((F1, HID), dtype=np.float32)
    w1p[:IN_F] = np.asarray(W1, dtype=np.float32)
    w3p = np.zeros((HID, F3), dtype=np.float32)
    w3p[:, :OUT_F] = np.asarray(W3, dtype=np.float32)
    b3p = np.zeros((F3O,), dtype=np.float32)
    b3p[:OUT_F] = np.asarray(b3, dtype=np.float32)

    ident = np.eye(P, dtype=np.float32)

    msg1 = _build_msg1(pre, xs).astype(ml_dtypes.float8_e4m3fn)

    # one-hot segment matrices, built on host: bmat[b, p, g*128+q] =
    # (segar[b, p, g] == q), bf16.  [BATCHES, P, CHB*P] per core.
    qs = np.arange(P, dtype=np.float32)

    common = dict(
        w1=w1p.astype(ml_dtypes.bfloat16),
        w2=np.asarray(W2, dtype=np.float32).astype(ml_dtypes.bfloat16),
        w3=w3p.astype(ml_dtypes.bfloat16),
        b1b=np.broadcast_to(np.asarray(b1, np.float32), (P, HID)).copy(),
        b2b=np.broadcast_to(np.asarray(b2, np.float32), (P, HID)).copy(),
        b3b=np.broadcast_to(b3p, (P, F3O)).copy(),
        ident=ident,
    )
    in_maps = []
    for c in range(NCORES):
        m = dict(common)
        wi = pre["widx"][c]                         # [B, PIECES, P, WCOL]
        m["widx"] = np.ascontiguousarray(
            wi.transpose(0, 2, 1, 3).reshape(wi.shape[0], P, -1))
        seg = pre["segar"][c]                       # [BATCHES, P, CHB]
        bmat = (seg[:, :, :, None] == qs).astype(ml_dtypes.float8_e4m3fn)
        m["bmat"] = np.ascontiguousarray(
            bmat.reshape(seg.shape[0], P, -1))
        m["dinv_t"] = pre["dinv_t"][c]
        # core c's own staged rows (local padded row r -> staged id);
        # fold the second (dst) dinv factor for the self-loop term here
        r = np.arange(SHARD_PAD)
        s = r // SL
        sid = s * (NCORES * SL) + c * SL + (r % SL)
        dloc = np.zeros(SHARD_PAD, dtype=np.float32)
        dloc[:SHARD] = dinv[c * SHARD:(c + 1) * SHARD]
        m["xself"] = np.ascontiguousarray(
            (xs[sid].astype(np.float32) * dloc[:, None])
            .astype(ml_dtypes.bfloat16))
        m["msg1"] = msg1[c]
        in_maps.append(m)

    trace = os.environ.get("KERNEL_TRACE", "0") == "1"
    if trace:
        _ensure_ntff_hook()
    res = run_bass_kernel_spmd(nc, in_maps, list(range(NCORES)), trace=trace)
    if trace and res.exec_time_ns is not None:
        print(f"HW exec time: {res.exec_time_ns} ns")
    if trace and res.instructions_and_trace is not None:
        print(f"trace path: {res.instructions_and_trace[1]}")

    out = np.concatenate(
        [res.results[c]["out_shard"][:SHARD, :OUT_F] for c in range(NCORES)],
        axis=0)
    if os.environ.get("KERNEL_DEBUG2", "0") == "1":
        kernel.dbg_acc = np.concatenate(
            [res.results[c]["dbg_acc"][:SHARD] for c in range(NCORES)], axis=0)
    if os.environ.get("KERNEL_DEBUG", "0") == "1":
        kernel.dbg_h1 = np.concatenate(
            [res.results[c]["dbg_h1"][:SHARD] for c in range(NCORES)], axis=0)
        kernel.dbg_t3 = np.concatenate(
            [res.results[c]["dbg_t3"][:SHARD] for c in range(NCORES)], axis=0)
    return out.astype(np.float32)



# revision 72
# speedup vs baseline: 1.1439x; 1.1439x over previous
"""GCN 3-layer kernel for Trainium2, 8 NeuronCores.

Full inputs in, full output out. Internally: dst-sharded SpMM via dma_gather
(bf16 tables, int16 window-relative indices) + one-hot matmul segment sum,
AllGather between layers, dense transforms per dst tile.
"""
import sys
sys.path.insert(0, "/opt/trn_rl_repo")
import os
import numpy as np
import ml_dtypes

import concourse.bass as bass
import concourse.bacc as bacc
import concourse.mybir as mybir
import concourse.tile as tile
from concourse.bass_utils import run_bass_kernel_spmd

P = 128
NCORES = 8
N_NODES = 100000
SHARD = N_NODES // NCORES           # 12500
TILES = (SHARD + P - 1) // P        # 98
SHARD_PAD = TILES * P               # 12544
PAD = SHARD_PAD - SHARD             # 44
NSTAGED = NCORES * SHARD_PAD        # 100352
NSLICE = 2                          # AllGather slices (7 batches each)
SL = SHARD_PAD // NSLICE            # 6272 rows per slice per core
IN_F, HID, OUT_F = 50, 256, 121
F1 = 64                             # x' padded width (fp8 msg, 64B rows)
F2 = 256                            # h1' width (fp8, 256B rows)
F3 = 256                            # t3' padded table width (fp8, 256B rows)
F3O = 128                           # L3 accumulator / output staging width
PIECES = 4
TB = 7                              # tiles per gather batch
BATCHES = TILES // TB               # 14
# source windows over staged-id domain [0, 100352): width 32768 each
WBASE = [0, 22528, 45056, 67584]

bf16 = mybir.dt.bfloat16
f32 = mybir.dt.float32
f8 = mybir.dt.float8e4
i16 = mybir.dt.int16

_CACHE = {}


def _ensure_ntff_hook():
    """Provide antenv.axon_hooks if the image lacks it, so trace=True works."""
    try:
        from antenv.axon_hooks import get_axon_ntff_profile_hook  # noqa: F401
        return
    except ImportError:
        pass
    import types
    mod = types.ModuleType("antenv.axon_hooks")
    mod._hook = None

    def set_axon_ntff_profile_hook(h):
        mod._hook = h

    def get_axon_ntff_profile_hook():
        return mod._hook

    mod.set_axon_ntff_profile_hook = set_axon_ntff_profile_hook
    mod.get_axon_ntff_profile_hook = get_axon_ntff_profile_hook
    sys.modules["antenv.axon_hooks"] = mod
    try:
        import antenv
        antenv.axon_hooks = mod
    except ImportError:
        pass
    try:
        from trn_agent_boot.trn_boot import _ntff_profile_via_ctypes
        h = _ntff_profile_via_ctypes("/opt/axon/libaxon_pjrt.so")
        if h is not None:
            mod._hook = h
    except Exception:
        pass


def _staged(v):
    """node id -> staged table id, slice-major so AllGather slices land
    contiguously: [slice][core][row-within-slice]."""
    c = v // SHARD
    r = v % SHARD
    s = r // SL
    return s * (NCORES * SL) + c * SL + (r % SL)


def _preprocess(edge_index):
    """Build per-core gather/segment streams with variable per-(tile,piece)
    chunk capacities (core-uniform). Returns dict of host arrays + plan."""
    src = np.asarray(edge_index[0], dtype=np.int64)
    dst = np.asarray(edge_index[1], dtype=np.int64)
    deg = (np.bincount(dst, minlength=N_NODES) + 1).astype(np.float64)
    dinv = (1.0 / np.sqrt(deg)).astype(np.float32)

    # per (core, tile): sorted source list + seg values
    ss_all = {}
    sg_all = {}
    e_ct = np.zeros((NCORES, TILES), dtype=np.int64)
    for c in range(NCORES):
        base = c * SHARD
        m = (dst >= base) & (dst < base + SHARD)
        sp = _staged(src[m])
        dl = dst[m] - base
        tl = dl >> 7
        seg = (dl & 127).astype(np.float32)
        key = tl * (1 << 17) + sp
        o = np.argsort(key, kind="stable")
        sp, seg, tl = sp[o], seg[o], tl[o]
        tcnt = np.bincount(tl, minlength=TILES)
        toff = np.concatenate([[0], np.cumsum(tcnt)])
        for t in range(TILES):
            ss_all[c, t] = sp[toff[t]:toff[t + 1]]
            sg_all[c, t] = seg[toff[t]:toff[t + 1]]
            e_ct[c, t] = toff[t + 1] - toff[t]

    # core-uniform per-(tile,piece) capacities n_tk (chunks of 128 slots)
    def cuts_for(ss, n_tk):
        cap = [P * n for n in n_tk]
        e = len(ss)
        cuts = [0]
        for k in range(PIECES):
            if k == PIECES - 1:
                nxt = e
            else:
                lo = int(np.searchsorted(ss, WBASE[k + 1]))
                hi = int(np.searchsorted(ss, WBASE[k] + 32768))
                need = e - sum(cap[k + 1:])
                nxt = min(hi, max(lo, need, cuts[-1]), cuts[-1] + cap[k])
                nxt = max(nxt, need)
                if nxt > hi or nxt < lo or nxt < cuts[-1]:
                    return None
            if nxt - cuts[-1] > cap[k]:
                return None
            cuts.append(nxt)
        return cuts

    n_tk_all = []
    for t in range(TILES):
        ct = int(-(-e_ct[:, t].max() // P))
        while True:
            bn, rem = ct // PIECES, ct % PIECES
            n_tk = [bn + (k < rem) for k in range(PIECES)]
            if all(cuts_for(ss_all[c, t], n_tk) is not None
                   for c in range(NCORES)):
                break
            ct += 1
        n_tk_all.append(n_tk)

    # per (batch, piece): stream length; per (batch): chunk column layout
    nk_bk = [[sum(n_tk_all[b * TB + ti][k] for ti in range(TB))
              for k in range(PIECES)] for b in range(BATCHES)]
    chb_b = [sum(nk) for nk in nk_bk]
    CHB = max(chb_b)
    WCOL = max(max(nk) for nk in nk_bk) * P // 16
    # column index of chunk (b, k, ti, cc) in the per-batch stream
    cols = []  # cols[b][ti] = list of column indices (piece-major)
    for b in range(BATCHES):
        offk = np.concatenate([[0], np.cumsum(nk_bk[b])])
        bt = []
        for ti in range(TB):
            t = b * TB + ti
            cl = []
            for k in range(PIECES):
                pre = sum(n_tk_all[b * TB + tj][k] for tj in range(ti))
                for cc in range(n_tk_all[t][k]):
                    cl.append(int(offk[k]) + pre + cc)
            bt.append(cl)
        cols.append(bt)

    widx = np.zeros((NCORES, BATCHES, PIECES, P, WCOL), dtype=np.int16)
    segar = np.full((NCORES, BATCHES, P, CHB), -1.0, dtype=np.float32)
    sp_slots = np.zeros((NCORES, BATCHES, CHB * P), dtype=np.int64)
    dinv_t = np.ones((NCORES, P, TILES), dtype=np.float32)
    for c in range(NCORES):
        base = c * SHARD
        for b in range(BATCHES):
            offk = np.concatenate([[0], np.cumsum(nk_bk[b])])
            for k in range(PIECES):
                nk = nk_bk[b][k]
                stream = np.zeros(nk * P, dtype=np.int16)
                segstr = np.full((nk, P), -1.0, dtype=np.float32)
                spstr = np.full(nk * P, WBASE[k], dtype=np.int64)
                pos = 0
                for ti in range(TB):
                    t = b * TB + ti
                    ss = ss_all[c, t]
                    sg = sg_all[c, t]
                    cuts = cuts_for(ss, n_tk_all[t])
                    a, bb = cuts[k], cuts[k + 1]
                    n = bb - a
                    rel = ss[a:bb] - WBASE[k]
                    assert (rel >= 0).all() and (rel < 32768).all()
                    cap = n_tk_all[t][k] * P
                    stream[pos:pos + n] = rel.astype(np.int16)
                    spstr[pos:pos + n] = ss[a:bb]
                    fl = segstr.reshape(-1)
                    fl[pos:pos + n] = sg[a:bb]
                    pos += cap
                w = stream.reshape(-1, 16).T
                widx[c, b, k, :, :nk * P // 16] = np.tile(w, (8, 1))
                segar[c, b, :, offk[k]:offk[k + 1]] = segstr.T
                sp_slots[c, b, offk[k] * P:offk[k + 1] * P] = spstr
        for t in range(TILES):
            lo = t * P
            n = max(0, min(P, SHARD - lo))
            if n > 0:
                dinv_t[c, :n, t] = dinv[base + lo:base + lo + n]
    plan = dict(nk_bk=nk_bk, chb_b=chb_b, CHB=CHB, WCOL=WCOL, cols=cols)
    return dict(widx=widx, segar=segar, dinv_t=dinv_t, dinv=dinv,
                sp_slots=sp_slots, plan=plan)


def _build_msg1(pre, xs):
    """Host pre-gather of the layer-1 message stream: slot i of batch b lands
    at msg1[b][i%128, i//128] (dma_gather output layout)."""
    CHB = pre["plan"]["CHB"]
    sp = pre["sp_slots"]  # [NCORES, BATCHES, CHB*P]
    msg1 = np.zeros((NCORES, BATCHES, P, CHB, F1), dtype=xs.dtype)
    for c in range(NCORES):
        for b in range(BATCHES):
            rows = xs[sp[c, b]]  # [CHB*P, F1]
            msg1[c, b] = rows.reshape(CHB, P, F1).transpose(1, 0, 2)
    return msg1


def _build_program(plan):
    """Build the (core-uniform) Bass program from the chunk plan."""
    nbatch = int(os.environ.get("KERNEL_NBATCH", str(BATCHES)))
    CH_B = plan["CHB"]
    WCOL = plan["WCOL"]
    nk_bk = plan["nk_bk"]
    cols = plan["cols"]

    nq = int(os.environ.get("KERNEL_NQ", "1"))
    nc = bacc.Bacc("TRN2", target_bir_lowering=False, debug=False,
                   enable_asserts=False, num_devices=NCORES,
                   num_swdge_queues=nq)

    t_msg1 = nc.dram_tensor("msg1", [BATCHES, P, CH_B, F1], f8,
                            kind="ExternalInput")
    t_widx = nc.dram_tensor("widx", [BATCHES, P, PIECES * WCOL], i16,
                            kind="ExternalInput")
    t_bmat = nc.dram_tensor("bmat", [BATCHES, P, CH_B * P], f8,
                            kind="ExternalInput")
    t_dinv = nc.dram_tensor("dinv_t", [P, TILES], f32, kind="ExternalInput")
    t_w1 = nc.dram_tensor("w1", [F1, HID], bf16, kind="ExternalInput")
    t_w2 = nc.dram_tensor("w2", [HID, HID], bf16, kind="ExternalInput")
    t_w3 = nc.dram_tensor("w3", [HID, F3], bf16, kind="ExternalInput")
    t_b1 = nc.dram_tensor("b1b", [P, HID], f32, kind="ExternalInput")
    t_b2 = nc.dram_tensor("b2b", [P, HID], f32, kind="ExternalInput")
    t_b3 = nc.dram_tensor("b3b", [P, F3O], f32, kind="ExternalInput")
    t_ident = nc.dram_tensor("ident", [P, P], f32, kind="ExternalInput")
    t_xselfT = nc.dram_tensor("xselfT", [F1, SHARD_PAD], bf16,
                              kind="ExternalInput")
    t_out = nc.dram_tensor("out_shard", [SHARD_PAD, F3O], f32,
                           kind="ExternalOutput")
    dbg = os.environ.get("KERNEL_DEBUG", "0") == "1"
    if dbg:
        t_dbg1 = nc.dram_tensor("dbg_h1", [SHARD_PAD, F2], f32,
                                kind="ExternalOutput")
        t_dbg3 = nc.dram_tensor("dbg_t3", [SHARD_PAD, F3O], f32,
                                kind="ExternalOutput")

    with tile.TileContext(nc) as tc:
        with (
            tc.tile_pool(name="consts", bufs=1) as consts,
            tc.tile_pool(name="stream", bufs=3) as stream,
            tc.tile_pool(name="msgp", bufs=4) as msgp,
            tc.tile_pool(name="work", bufs=3) as work,
            tc.tile_pool(name="bpool", bufs=2) as bpool,
            tc.tile_pool(name="psum", bufs=3, space="PSUM") as psum,
            tc.tile_pool(name="psumd", bufs=2, space="PSUM") as psumd,
            tc.tile_pool(name="dram", bufs=1, space="DRAM") as dram,
        ):
            ident_t = consts.tile([P, P], f32)
            nc.sync.dma_start(out=ident_t[:], in_=t_ident[:])
            dinv_c = consts.tile([P, TILES], f32)
            nc.sync.dma_start(out=dinv_c[:], in_=t_dinv[:])
            w1_t = consts.tile([F1, HID], bf16)
            nc.sync.dma_start(out=w1_t[:], in_=t_w1[:])
            w2_ts = []
            for kk in range(2):
                wt = consts.tile([P, HID], bf16, name=f"w2t{kk}")
                nc.sync.dma_start(out=wt[:], in_=t_w2[kk * P:(kk + 1) * P, :])
                w2_ts.append(wt)
            w3_ts = []
            for kk in range(2):
                wt = consts.tile([P, F3], bf16, name=f"w3t{kk}")
                nc.sync.dma_start(out=wt[:], in_=t_w3[kk * P:(kk + 1) * P, :])
                w3_ts.append(wt)
            b1_t = consts.tile([P, HID], f32)
            nc.sync.dma_start(out=b1_t[:], in_=t_b1[:])
            b2_t = consts.tile([P, HID], f32)
            nc.sync.dma_start(out=b2_t[:], in_=t_b2[:])
            b3_t = consts.tile([P, F3O], f32)
            nc.sync.dma_start(out=b3_t[:], in_=t_b3[:])

            h1_stage = dram.tile([SHARD_PAD, F2], f8)
            h1_full = dram.tile([NSTAGED, F2], f8)
            t3_stage = dram.tile([SHARD_PAD, F3], f8)
            t3_full = dram.tile([NSTAGED, F3], f8)
            # per-slice AllGather landing buffers (Shared = peer-writable;
            # each is written by exactly one collective)
            ag_h1 = [dram.tile([SL * NCORES, F2], f8, addr_space="Shared",
                               name=f"agh1_{s}") for s in range(NSLICE)]
            ag_t3 = [dram.tile([SL * NCORES, F3], f8, addr_space="Shared",
                               name=f"agt3_{s}") for s in range(NSLICE)]

            def spmm_layer(layer, table_ap, elem, accw, tail_fn,
                           after_batch=None):
                # layer 1 runs "transposed": messages are the stationary
                # operand, acc comes out feature-major [F1, 128 nodes],
                # which feeds the dense transform without a transpose.
                use_dr = True
                dr = mybir.MatmulPerfMode.DoubleRow
                for b in range(nbatch):
                    offk = [0]
                    for k in range(PIECES):
                        offk.append(offk[-1] + nk_bk[b][k])

                    def piece_of(g):
                        kg = 0
                        while kg < PIECES - 1 and g >= offk[kg + 1]:
                            kg += 1
                        return kg

                    bm = bpool.tile([P, CH_B * P], f8, tag="bm")
                    nc.sync.dma_start(out=bm[:], in_=t_bmat[b])
                    msg = msgp.tile([P, CH_B, elem], f8, tag="msg")
                    if layer == 1:
                        nc.sync.dma_start(out=msg[:], in_=t_msg1[b])
                    else:
                        it = stream.tile([P, PIECES * WCOL], i16, tag="idx")
                        nc.sync.dma_start(out=it[:], in_=t_widx[b])
                        sp_mode = os.environ.get("KERNEL_SP", "0") == "1"
                        for k in range(PIECES):
                            nk = nk_bk[b][k]
                            nc.gpsimd.dma_gather(
                                msg[:, offk[k]:offk[k + 1], :],
                                table_ap[WBASE[k]:WBASE[k] + 32768, :],
                                it[:, k * WCOL:k * WCOL + nk * P // 16],
                                nk * P, nk * P, elem,
                                single_packet=sp_mode,
                                queue_num=k % nq,
                            )
                    for ti in range(TB):
                        t = b * TB + ti
                        cl = cols[b][ti]
                        if layer == 1:
                            acc = psum.tile([P, P], f32, tag="tp")
                        else:
                            acc = psum.tile([P, accw], f32, tag="acc")
                        items = []      # (g, pair)
                        q = 0
                        while q < len(cl):
                            g = cl[q]
                            if (use_dr and q + 1 < len(cl)
                                    and cl[q + 1] == g + 1):
                                items.append((g, True))
                                q += 2
                            else:
                                items.append((g, False))
                                q += 1
                        for j, (g, pair) in enumerate(items):
                            st_ = (j == 0)
                            sp_ = (j == len(items) - 1)
                            if layer == 1:
                                if pair:
                                    nc.tensor.matmul(
                                        out=acc[:accw, :],
                                        lhsT=msg[:, g:g + 2, :],
                                        rhs=bm[:, g * P:(g + 2) * P]
                                        .rearrange("p (k m) -> p k m", k=2),
                                        start=st_, stop=sp_, perf_mode=dr)
                                else:
                                    nc.tensor.matmul(
                                        out=acc[:accw, :],
                                        lhsT=msg[:, g:g + 1, :],
                                        rhs=bm[:, g * P:(g + 1) * P],
                                        start=st_, stop=sp_)
                            elif pair:
                                nc.tensor.matmul(
                                    out=acc[:],
                                    lhsT=bm[:, g * P:(g + 2) * P]
                                    .rearrange("p (k m) -> p k m", k=2),
                                    rhs=msg[:, g:g + 2, :accw],
                                    start=st_, stop=sp_, perf_mode=dr)
                            else:
                                nc.tensor.matmul(
                                    out=acc[:],
                                    lhsT=bm[:, g * P:(g + 1) * P],
                                    rhs=msg[:, g:g + 1, :accw],
                                    start=st_, stop=sp_)
                        tail_fn(t, acc)
                    if after_batch is not None:
                        after_batch(b)

            def dense(lhs_sbuf_f32, wts, fout, kw=P):
                """lhs [P, nk*kw] f32 sbuf (node rows) -> psum [P, fout]"""
                nk = len(wts)
                o2 = psumd.tile([P, fout], f32, tag="dense")
                for kk in range(nk):
                    tp = psum.tile([P, P], f32, tag="tp")
                    nc.tensor.transpose(
                        out=tp[:kw, :],
                        in_=lhs_sbuf_f32[:, kk * kw:(kk + 1) * kw],
                        identity=ident_t[:])
                    lt = work.tile([P, P], bf16, tag="lt")
                    nc.scalar.activation(
                        out=lt[:kw, :], in_=tp[:kw, :],
                        func=mybir.ActivationFunctionType.Copy)
                    nc.tensor.matmul(
                        out=o2[:], lhsT=lt[:kw, :], rhs=wts[kk][:, :fout],
                        start=(kk == 0), stop=(kk == nk - 1))
                return o2

            def tail1(t, acc):
                # acc[:F1, :] is feature-major [F1, 128 nodes]
                st = work.tile([F1, P], bf16, tag="selft")
                nc.sync.dma_start(out=st[:],
                                  in_=t_xselfT[:, t * P:(t + 1) * P])
                agg0 = work.tile([F1, P], f32, tag="agg0")
                nc.vector.tensor_tensor(
                    out=agg0[:], in0=st[:], in1=acc[:F1, :],
                    op=mybir.AluOpType.add)
                lt = work.tile([F1, P], bf16, tag="lt1")
                nc.scalar.activation(
                    out=lt[:], in_=agg0[:],
                    func=mybir.ActivationFunctionType.Copy)
                o2 = psumd.tile([P, HID], f32, tag="dense")
                nc.tensor.matmul(out=o2[:], lhsT=lt[:], rhs=w1_t[:, :HID],
                                 start=True, stop=True)
                s1 = work.tile([P, HID], f32, tag="s1")
                nc.vector.scalar_tensor_tensor(
                    out=s1[:], in0=o2[:], scalar=dinv_c[:, t:t + 1],
                    in1=b1_t[:], op0=mybir.AluOpType.mult,
                    op1=mybir.AluOpType.add)
                h1t = work.tile([P, HID], f8, tag="h1t")
                nc.scalar.activation(
                    out=h1t[:], in_=s1[:],
                    func=mybir.ActivationFunctionType.Relu,
                    scale=dinv_c[:, t:t + 1])
                nc.sync.dma_start(out=h1_stage[t * P:(t + 1) * P, :], in_=h1t[:])
                if dbg:
                    h1f = work.tile([P, HID], f32, tag="h1f")
                    nc.vector.tensor_copy(out=h1f[:], in_=h1t[:])
                    nc.sync.dma_start(out=t_dbg1[t * P:(t + 1) * P, :],
                                      in_=h1f[:])

            def tail2(t, acc):
                st8 = work.tile([P, HID], f8, tag="selft2")
                nc.sync.dma_start(out=st8[:],
                                  in_=h1_stage[t * P:(t + 1) * P, :])
                st = work.tile([P, HID], f32, tag="selfc2")
                nc.scalar.activation(
                    out=st[:], in_=st8[:],
                    func=mybir.ActivationFunctionType.Copy,
                    scale=dinv_c[:, t:t + 1])
                agg0 = work.tile([P, HID], f32, tag="agg02")
                nc.vector.scalar_tensor_tensor(
                    out=agg0[:], in0=acc[:], scalar=dinv_c[:, t:t + 1],
                    in1=st[:], op0=mybir.AluOpType.mult,
                    op1=mybir.AluOpType.add)
                o2 = dense(agg0, w2_ts, HID)
                s2 = work.tile([P, HID], f32, tag="s1")
                nc.vector.tensor_tensor(out=s2[:], in0=o2[:], in1=b2_t[:],
                                        op=mybir.AluOpType.add)
                h2t = work.tile([P, HID], f32, tag="h2t")
                nc.scalar.activation(
                    out=h2t[:], in_=s2[:],
                    func=mybir.ActivationFunctionType.Relu,
                    scale=dinv_c[:, t:t + 1])
                o3 = dense(h2t, w3_ts, F3)
                t3t = work.tile([P, F3], f8, tag="t3t")
                nc.scalar.activation(
                    out=t3t[:], in_=o3[:],
                    func=mybir.ActivationFunctionType.Copy)
                nc.sync.dma_start(out=t3_stage[t * P:(t + 1) * P, :], in_=t3t[:])
                if dbg:
                    t3f = work.tile([P, F3O], f32, tag="t3f")
                    nc.vector.tensor_copy(out=t3f[:], in_=t3t[:, :F3O])
                    nc.sync.dma_start(out=t_dbg3[t * P:(t + 1) * P, :],
                                      in_=t3f[:])

            def tail3(t, acc):
                st8 = work.tile([P, F3O], f8, tag="selft3")
                nc.sync.dma_start(
                    out=st8[:], in_=t3_stage[t * P:(t + 1) * P, :F3O])
                st = work.tile([P, F3O], f32, tag="selfc3")
                nc.scalar.activation(
                    out=st[:], in_=st8[:],
                    func=mybir.ActivationFunctionType.Copy,
                    scale=dinv_c[:, t:t + 1])
                agg0 = work.tile([P, F3O], f32, tag="agg03")
                nc.vector.scalar_tensor_tensor(
                    out=agg0[:], in0=acc[:], scalar=dinv_c[:, t:t + 1],
                    in1=st[:], op0=mybir.AluOpType.mult,
                    op1=mybir.AluOpType.add)
                res = work.tile([P, F3O], f32, tag="res")
                nc.vector.tensor_tensor(
                    out=res[:], in0=agg0[:], in1=b3_t[:],
                    op=mybir.AluOpType.add)
                nc.sync.dma_start(out=t_out[t * P:(t + 1) * P, :], in_=res[:])

            nlayer = int(os.environ.get("KERNEL_NLAYER", "3"))

            def slice_ag(stage, full, bufs):
                bps = BATCHES // NSLICE
                def after_batch(b):
                    if (b + 1) % bps == 0:
                        s = (b + 1) // bps - 1
                        nc.gpsimd.collective_compute(
                            "AllGather", mybir.AluOpType.bypass,
                            replica_groups=[list(range(NCORES))],
                            ins=[stage[s * SL:(s + 1) * SL, :].opt()],
                            outs=[bufs[s][:].opt()])
                        nc.sync.dma_start(
                            out=full[s * SL * NCORES:(s + 1) * SL * NCORES,
                                     :],
                            in_=bufs[s][:])
                return after_batch

            spmm_layer(1, None, F1, F1, tail1,
                       after_batch=(slice_ag(h1_stage, h1_full, ag_h1)
                                    if nlayer >= 2 else None))
            if nlayer >= 2:
                spmm_layer(2, h1_full, F2, F2, tail2,
                           after_batch=(slice_ag(t3_stage, t3_full, ag_t3)
                                        if nlayer >= 3 else None))
            if nlayer >= 3:
                spmm_layer(3, t3_full, F3, F3O, tail3)

    nc.compile()
    return nc


def kernel(x, edge_index, W1, b1, W2, b2, W3, b3):
    x = np.asarray(x, dtype=np.float32)
    pre = _preprocess(np.asarray(edge_index))
    plan = pre["plan"]
    key = tuple(tuple(nk) for nk in plan["nk_bk"])

    if key not in _CACHE:
        _CACHE[key] = _build_program(plan)
    nc = _CACHE[key]

    dinv = pre["dinv"]
    xs = np.zeros((NSTAGED, F1), dtype=np.float32)
    xp = dinv[:, None] * x                      # [N, 50]
    xs[_staged(np.arange(N_NODES)), :IN_F] = xp
    xs = xs.astype(ml_dtypes.bfloat16)

    w1p = np.zeros((F1, HID), dtype=np.float32)
    w1p[:IN_F] = np.asarray(W1, dtype=np.float32)
    w3p = np.zeros((HID, F3), dtype=np.float32)
    w3p[:, :OUT_F] = np.asarray(W3, dtype=np.float32)
    b3p = np.zeros((F3O,), dtype=np.float32)
    b3p[:OUT_F] = np.asarray(b3, dtype=np.float32)

    ident = np.eye(P, dtype=np.float32)

    msg1 = _build_msg1(pre, xs).astype(ml_dtypes.float8_e4m3fn)

    # one-hot segment matrices, built on host: bmat[b, p, g*128+q] =
    # (segar[b, p, g] == q), bf16.  [BATCHES, P, CHB*P] per core.
    qs = np.arange(P, dtype=np.float32)

    common = dict(
        w1=w1p.astype(ml_dtypes.bfloat16),
        w2=np.asarray(W2, dtype=np.float32).astype(ml_dtypes.bfloat16),
        w3=w3p.astype(ml_dtypes.bfloat16),
        b1b=np.broadcast_to(np.asarray(b1, np.float32), (P, HID)).copy(),
        b2b=np.broadcast_to(np.asarray(b2, np.float32), (P, HID)).copy(),
        b3b=np.broadcast_to(b3p, (P, F3O)).copy(),
        ident=ident,
    )
    in_maps = []
    for c in range(NCORES):
        m = dict(common)
        wi = pre["widx"][c]                         # [B, PIECES, P, WCOL]
        m["widx"] = np.ascontiguousarray(
            wi.transpose(0, 2, 1, 3).reshape(wi.shape[0], P, -1))
        seg = pre["segar"][c]                       # [BATCHES, P, CHB]
        bmat = (seg[:, :, :, None] == qs).astype(ml_dtypes.float8_e4m3fn)
        m["bmat"] = np.ascontiguousarray(
            bmat.reshape(seg.shape[0], P, -1))
        m["dinv_t"] = pre["dinv_t"][c]
        # core c's own staged rows (local padded row r -> staged id),
        # transposed to feature-major for the L1 transposed tail
        r = np.arange(SHARD_PAD)
        s = r // SL
        sid = s * (NCORES * SL) + c * SL + (r % SL)
        m["xselfT"] = np.ascontiguousarray(xs[sid].T)
        m["msg1"] = msg1[c]
        in_maps.append(m)

    trace = os.environ.get("KERNEL_TRACE", "0") == "1"
    if trace:
        _ensure_ntff_hook()
    res = run_bass_kernel_spmd(nc, in_maps, list(range(NCORES)), trace=trace)
    if trace and res.exec_time_ns is not None:
        print(f"HW exec time: {res.exec_time_ns} ns")
    if trace and res.instructions_and_trace is not None:
        print(f"trace path: {res.instructions_and_trace[1]}")

    out = np.concatenate(
        [res.results[c]["out_shard"][:SHARD, :OUT_F] for c in range(NCORES)],
        axis=0)
    if os.environ.get("KERNEL_DEBUG", "0") == "1":
        kernel.dbg_h1 = np.concatenate(
            [res.results[c]["dbg_h1"][:SHARD] for c in range(NCORES)], axis=0)
        kernel.dbg_t3 = np.concatenate(
            [res.results[c]["dbg_t3"][:SHARD] for c in range(NCORES)], axis=0)
    return out.astype(np.float32)

